# revision 2
# baseline (speedup 1.0000x reference)
"""Trainium2 Bass kernel v3 for nn_SinkhornLayer: 10 exp-domain Sinkhorn
iterations on 64 independent [1024,1024] fp32 matrices, batch-sharded over 8
NeuronCores (8 matrices/core).

Math (validated ~6.5e-3 relmax vs the log-domain reference):
    K  = exp(10*M - 40)  stored bf16
    u0 = 1/rowsum(K)     (fp32 accum fused into the ACT exp pass)
    iterate 10x:  v = 1/(K^T u)   u' = 1/(K v)    [u,v bf16, psum fp32]
    out = diag(u) K diag(v)  in fp32

v3 structure (vs v2): the PE half-sweeps run at ~1.87us warm when
back-to-back; v2 lost ~250us/core to PE stalls.  Fixes:
  - software-pipelined pairs: pair p+1's loads+exp (DMA sync queue + ACT)
    are emitted before pair p's iteration sweeps, so the PE never waits on
    HBM at pair boundaries (also keeps the PE HAM-warm at 2.4 GHz);
  - within an iteration the sweep order is u(m0),u(m1),v(m0),v(m1) so each
    DVE reciprocal hides under the other matrix's PE sweep;
  - output stores go on the SWDGE (gpsimd) queue so they never queue ahead
    of the next pair's input loads on the sync queue;
  - K^T is built by xbar DMA-transpose on the ACT hwdge queue (PH2_MODE
    "dma"), freeing ~4us/matrix of PE time; "pe" falls back to PE block
    transposes emitted after the previous pair's sweeps.
"""
import numpy as np
from contextlib import ExitStack

import concourse.bacc as bacc
import concourse.bass as bass
import concourse.tile as tile
from concourse import mybir
from concourse.bass_utils import run_bass_kernel_spmd
from concourse.masks import make_identity

F32 = mybir.dt.float32
BF16 = mybir.dt.bfloat16
AF = mybir.ActivationFunctionType
ALU = mybir.AluOpType

P = 128
N = 1024
B = 64
NCORES = 8
BPC = B // NCORES    # 8 matrices per core
TPM = N // P         # 8 tiles per matrix dim
ITERS = 10
INV_EPS = 10.0
SHIFT = 40.0         # bias in P0 domain: K = exp(10*m - 40)
PH2_MODE = "dma"     # "dma" (xbar DMATranspose on ACT queue) | "pe"


def _half_sweep(nc, pools, mat, w_bf, want_f32=False):
    """One Sinkhorn half-iteration: w_new = 1/(mat^T w) with mat given as
    [P, TPM, N] tiled blocks (contraction dim on partitions).  Returns
    (w_new bf16 [P,TPM], w_new fp32 or None)."""
    pssw, vec = pools["pssw"], pools["vec"]
    ps = pssw.tile([P, TPM], F32, tag="swc")
    for a in range(TPM):
        for c in range(TPM):
            nc.tensor.matmul(
                ps[:, a:a + 1],
                mat[:, c, a * P:(a + 1) * P],
                w_bf[:, c:c + 1],
                start=(c == 0), stop=(c == TPM - 1),
            )
    w_new = vec.tile([P, TPM], BF16, tag="wbf")
    nc.vector.reciprocal(w_new, ps)
    w32 = None
    if want_f32:
        w32 = vec.tile([P, TPM], F32, tag="w32")
        nc.vector.reciprocal(w32, ps)
    return w_new, w32


class _Pipe:
    """Per-core Sinkhorn pipeline over BPC matrices in pairs."""

    def __init__(self, ctx, tc, out_ap, m_ap, n_in):
        self.nc = tc.nc
        self.tc = tc
        self.out_ap = out_ap
        self.m_ap = m_ap
        self.n_in = n_in
        nc = self.nc
        const = ctx.enter_context(tc.tile_pool(name="const", bufs=1))
        self.ident_bf = const.tile([P, P], BF16)
        make_identity(nc, self.ident_bf[:])
        self.ones_row = const.tile([1, P], BF16)
        nc.vector.memset(self.ones_row, 1.0)
        self.negshift = const.tile([P, 1], F32)
        nc.vector.memset(self.negshift, -SHIFT)

        self.ppool = ctx.enter_context(tc.tile_pool(name="p0", bufs=3))
        self.ktp = ctx.enter_context(tc.tile_pool(name="kt", bufs=4))
        self.kttp = ctx.enter_context(tc.tile_pool(name="ktt", bufs=4))
        self.vec = ctx.enter_context(tc.tile_pool(name="vec", bufs=12))
        self.sbrow = ctx.enter_context(tc.tile_pool(name="sbrow", bufs=2))
        self.epool = ctx.enter_context(tc.tile_pool(name="eout", bufs=3))

        # PSUM budget (8 banks): pssw 3 + pstr 2 (pe mode) + ps4 1 + psr 1
        self.pssw = ctx.enter_context(
            tc.tile_pool(name="pssw", bufs=3, space="PSUM"))
        if PH2_MODE == "pe":
            self.pstr = ctx.enter_context(
                tc.tile_pool(name="pstr", bufs=2, space="PSUM"))
        self.ps4 = ctx.enter_context(
            tc.tile_pool(name="ps4", bufs=1, space="PSUM"))
        self.psr = ctx.enter_context(
            tc.tile_pool(name="psr", bufs=1, space="PSUM"))
        self.pools = {"pssw": self.pssw, "vec": self.vec}

    def ph1_load_exp(self, b):
        """DMA load (sync queue) + K = exp(10*m - 40) bf16 + fused rowsum."""
        nc = self.nc
        kt = self.ktp.tile([P, TPM, N], BF16, tag="kt")
        rowsum = self.vec.tile([P, TPM], F32, tag="rs")
        for ti in range(TPM):
            p0 = self.ppool.tile([P, N], F32, tag="p0")
            nc.sync.dma_start(
                out=p0, in_=self.m_ap[b % self.n_in, ti * P:(ti + 1) * P, :])
            nc.scalar.activation(out=kt[:, ti, :], in_=p0, func=AF.Exp,
                                 bias=self.negshift[:, 0:1], scale=INV_EPS,
                                 accum_out=rowsum[:, ti:ti + 1])
        u_bf = self.vec.tile([P, TPM], BF16, tag="ubf")
        nc.vector.reciprocal(u_bf, rowsum)
        return dict(b=b, kt=kt, u=u_bf)

    def ph2_transpose(self, m):
        """Build K^T blocks."""
        nc = self.nc
        ktt = self.kttp.tile([P, TPM, N], BF16, tag="ktt")
        if PH2_MODE == "dma":
            for tj in range(TPM):
                for ti in range(TPM):
                    nc.scalar.dma_start_transpose(
                        out=ktt[:, tj, ti * P:(ti + 1) * P],
                        in_=m["kt"][:, ti, tj * P:(tj + 1) * P])
        else:
            for tj in range(TPM):
                pt = self.pstr.tile([P, N], BF16, tag="pt")
                for ti in range(TPM):
                    nc.tensor.transpose(pt[:, ti * P:(ti + 1) * P],
                                        m["kt"][:, ti, tj * P:(tj + 1) * P],
                                        self.ident_bf)
                if tj % 2 == 0:
                    nc.vector.tensor_copy(ktt[:, tj, :], pt)
                else:
                    nc.scalar.copy(ktt[:, tj, :], pt)
        m["ktt"] = ktt

    def ph3_iterate(self, ms):
        """10 Sinkhorn iterations, pair-interleaved: all u-sweeps of the pair
        first, then all v-sweeps, so each reciprocal hides under the other
        matrix's PE sweep."""
        nc = self.nc
        for t in range(ITERS):
            if t > 0:
                want32 = (t == ITERS - 1)
                for m in ms:
                    u_bf, u32 = _half_sweep(nc, self.pools, m["ktt"], m["v"],
                                            want_f32=want32)
                    if u32 is not None:
                        m["u32"] = u32
                    m["u"] = u_bf
            for m in ms:
                m["v"], _ = _half_sweep(nc, self.pools, m["kt"], m["u"])

    def ph4_output(self, m):
        """out = diag(u32) K diag(v); stores on the SWDGE (gpsimd) queue."""
        nc = self.nc
        if "u32" not in m:
            m["u32"] = m["u"]
        vr_ps = self.psr.tile([1, N], BF16, tag="vr", bufs=1)
        for tj in range(TPM):
            nc.tensor.transpose(vr_ps[0:1, tj * P:(tj + 1) * P],
                                m["v"][:, tj:tj + 1], self.ident_bf)
        vrow = self.sbrow.tile([1, N], BF16, tag="vrow")
        nc.vector.tensor_copy(vrow, vr_ps)
        vb = self.ps4.tile([P, N], F32, tag="vb")
        for h in range(2):
            nc.tensor.matmul(vb[:, h * 512:(h + 1) * 512], self.ones_row,
                             vrow[0:1, h * 512:(h + 1) * 512],
                             start=True, stop=True)
        for ti in range(TPM):
            e = self.epool.tile([P, N], F32, tag="e")
            nc.vector.scalar_tensor_tensor(
                out=e, in0=m["kt"][:, ti, :], scalar=m["u32"][:, ti:ti + 1],
                in1=vb, op0=ALU.mult, op1=ALU.mult)
            nc.gpsimd.dma_start(out=self.out_ap[m["b"], ti * P:(ti + 1) * P, :],
                                in_=e)


def sinkhorn_kernel(ctx, tc, out_ap, m_ap, n_in=None):
    nc = tc.nc
    if n_in is None:
        n_in = BPC
    ctx.enter_context(nc.allow_low_precision(
        reason="bf16 u/v iterates validated to 6.5e-3 relmax vs fp64 ref"))
    pipe = _Pipe(ctx, tc, out_ap, m_ap, n_in)

    npairs = BPC // 2
    pairs = [(2 * pr, 2 * pr + 1) for pr in range(npairs)]
    live = [pipe.ph1_load_exp(b) for b in pairs[0]]
    for m in live:
        pipe.ph2_transpose(m)
    for pr in range(npairs):
        nxt = None
        if pr + 1 < npairs:
            # prefetch next pair: loads+exp now; transposes too when they
            # don't touch the PE (dma mode)
            nxt = [pipe.ph1_load_exp(b) for b in pairs[pr + 1]]
            if PH2_MODE == "dma":
                for m in nxt:
                    pipe.ph2_transpose(m)
        pipe.ph3_iterate(live)
        if nxt is not None and PH2_MODE == "pe":
            for m in nxt:
                pipe.ph2_transpose(m)
        for m in live:
            pipe.ph4_output(m)
        live = nxt


_CACHE = {}


def _build():
    if "main" in _CACHE:
        return _CACHE["main"]
    nc = bacc.Bacc("TRN2", target_bir_lowering=False, debug=False,
                   num_devices=NCORES)
    m_ap = nc.dram_tensor("m", [BPC, N, N], F32, kind="ExternalInput").ap()
    out_ap = nc.dram_tensor("out", [BPC, N, N], F32, kind="ExternalOutput").ap()
    with tile.TileContext(nc) as tc:
        with ExitStack() as ctx:
            sinkhorn_kernel(ctx, tc, out_ap, m_ap)
    nc.compile()
    _CACHE["main"] = nc
    return nc


def kernel(M: np.ndarray) -> np.ndarray:
    M = np.ascontiguousarray(M, dtype=np.float32)
    assert M.shape == (B, N, N)
    nc = _build()
    in_maps = [{"m": M[c * BPC:(c + 1) * BPC]} for c in range(NCORES)]
    res = run_bass_kernel_spmd(nc, in_maps, core_ids=list(range(NCORES)))
    return np.concatenate([res.results[c]["out"] for c in range(NCORES)], axis=0)


N_IN_TIMING = 2   # aliased timing input matrices (keeps host->device at 8MB)


def _build_timing(loop_n):
    """Timing NEFF: full per-core workload in a hardware For_i loop.
    Input is a small aliased [N_IN_TIMING,N,N] tensor and the output goes to
    internal DRAM scratch, so host<->device transfer noise stays tiny while
    device-side DMA volume per rep is identical to the real kernel."""
    key = ("timing", loop_n)
    if key in _CACHE:
        return _CACHE[key]
    nc = bacc.Bacc("TRN2", target_bir_lowering=False, debug=False,
                   num_devices=NCORES)
    m_ap = nc.dram_tensor("m", [N_IN_TIMING, N, N], F32,
                          kind="ExternalInput").ap()
    out_ap = nc.dram_tensor("oscr", [BPC, N, N], F32, kind="Internal").ap()
    sink_ap = nc.dram_tensor("sink", [P, TPM], F32, kind="ExternalOutput").ap()
    with tile.TileContext(nc) as tc:
        with ExitStack() as ctx:
            with tc.For_i(0, loop_n, 1):
                sinkhorn_kernel(ctx, tc, out_ap, m_ap, n_in=N_IN_TIMING)
        # one tiny real output so the PJRT executable has something to fetch
        with ExitStack() as ctx2:
            pool = ctx2.enter_context(tc.tile_pool(name="snk", bufs=1))
            t = pool.tile([P, TPM], F32)
            nc.vector.memset(t, 1.0)
            nc.sync.dma_start(out=sink_ap, in_=t)
    nc.compile()
    _CACHE[key] = nc
    return nc


def time_hw(lo=100, hi=1100, runs=6, M=None):
    """Per-rep device ns via hardware-loop delta (one rep = full per-core
    workload of BPC matrices; all 8 cores run the same thing in parallel)."""
    import time as _time
    if M is None:
        rng = np.random.default_rng(7)
        M = rng.standard_normal((N_IN_TIMING, N, N), dtype=np.float32)
    in_maps = [{"m": M} for _ in range(NCORES)]
    # Interleave lo/hi samples so tunnel-congestion drift hits both phases
    # equally instead of inflating one side of the delta.
    ncs = {n: _build_timing(n) for n in (lo, hi)}
    for n in (lo, hi):
        run_bass_kernel_spmd(ncs[n], in_maps, core_ids=list(range(NCORES)))
    walls = {lo: [], hi: []}
    for _ in range(runs):
        for n in (lo, hi):
            t0 = _time.time()
            run_bass_kernel_spmd(ncs[n], in_maps, core_ids=list(range(NCORES)))
            walls[n].append(_time.time() - t0)
    for n in (lo, hi):
        print(f"loop_n={n}: walls={[f'{w:.3f}' for w in walls[n]]}", flush=True)
    t = (min(walls[hi]) - min(walls[lo])) / (hi - lo)
    return t * 1e9, walls


# revision 8
# speedup vs baseline: 2.6268x; 2.6268x over previous
"""Trainium2 Bass kernel v3 for nn_SinkhornLayer: 10 exp-domain Sinkhorn
iterations on 64 independent [1024,1024] fp32 matrices, batch-sharded over 8
NeuronCores (8 matrices/core).

Math (validated ~6.5e-3 relmax vs the log-domain reference):
    K  = exp(10*M - 40)  stored bf16
    u0 = 1/rowsum(K)     (fp32 accum fused into the ACT exp pass)
    iterate 10x:  v = 1/(K^T u)   u' = 1/(K v)    [u,v bf16, psum fp32]
    out = diag(u) K diag(v)  in fp32

v3 structure (vs v2):
  - software-pipelined pairs: pair p+1's loads+exp (DMA sync queue + ACT)
    are emitted before pair p's iteration sweeps, so the PE never waits on
    HBM at pair boundaries (also keeps the PE HAM-warm at 2.4 GHz);
  - within an iteration the sweep order is u(m0),u(m1),v(m0),v(m1) so each
    DVE reciprocal hides under the other matrix's PE sweep;
  - output stores go on the SWDGE (gpsimd) queue so they never queue ahead
    of the next pair's input loads on the sync queue;
  - K^T built by PE block transposes emitted after the previous pair's
    sweeps (DMA-transpose mode measured 2.5x slower end-to-end).
"""
import numpy as np
from contextlib import ExitStack

import concourse.bacc as bacc
import concourse.bass as bass
import concourse.tile as tile
from concourse import mybir
from concourse.bass_utils import run_bass_kernel_spmd
from concourse.masks import make_identity

F32 = mybir.dt.float32
BF16 = mybir.dt.bfloat16
AF = mybir.ActivationFunctionType
ALU = mybir.AluOpType

P = 128
N = 1024
B = 64
NCORES = 8
BPC = B // NCORES    # 8 matrices per core
TPM = N // P         # 8 tiles per matrix dim
ITERS = 10
INV_EPS = 10.0
SHIFT = 40.0         # bias in P0 domain: K = exp(10*m - 40)
PH2_MODE = "pe"      # "pe" (block transposes) | "dma" (xbar, slower)
STORE_Q = "gpsimd"   # "gpsimd" | "sync" | "scalar"
ABL_LOAD = True      # ablation knobs (timing experiments only)
ABL_EXP = True
ABL_PH2 = True
ABL_PH4 = True


def _half_sweep(nc, pools, mat, w_bf, want_f32=False):
    """One Sinkhorn half-iteration: w_new = 1/(mat^T w) with mat given as
    [P, TPM, N] tiled blocks (contraction dim on partitions).  Returns
    (w_new bf16 [P,TPM], w_new fp32 or None)."""
    pssw, vec = pools["pssw"], pools["vec"]
    ps = pssw.tile([P, TPM], F32, tag="swc")
    for a in range(TPM):
        for c in range(TPM):
            nc.tensor.matmul(
                ps[:, a:a + 1],
                mat[:, c, a * P:(a + 1) * P],
                w_bf[:, c:c + 1],
                start=(c == 0), stop=(c == TPM - 1),
            )
    w_new = vec.tile([P, TPM], BF16, tag="wbf")
    nc.vector.reciprocal(w_new, ps)
    w32 = None
    if want_f32:
        w32 = vec.tile([P, TPM], F32, tag="w32")
        nc.vector.reciprocal(w32, ps)
    return w_new, w32


class _Pipe:
    """Per-core Sinkhorn pipeline over BPC matrices in pairs."""

    def __init__(self, ctx, tc, out_ap, m_ap, n_in):
        self.nc = tc.nc
        self.tc = tc
        self.out_ap = out_ap
        self.m_ap = m_ap
        self.n_in = n_in
        nc = self.nc
        const = ctx.enter_context(tc.tile_pool(name="const", bufs=1))
        self.ident_bf = const.tile([P, P], BF16)
        make_identity(nc, self.ident_bf[:])
        self.ones_row = const.tile([1, P], BF16)
        nc.vector.memset(self.ones_row, 1.0)
        self.negshift = const.tile([P, 1], F32)
        nc.vector.memset(self.negshift, -SHIFT)

        self.ppool = ctx.enter_context(tc.tile_pool(name="p0", bufs=3))
        self.ktp = ctx.enter_context(tc.tile_pool(name="kt", bufs=4))
        self.kttp = ctx.enter_context(tc.tile_pool(name="ktt", bufs=4))
        self.vec = ctx.enter_context(tc.tile_pool(name="vec", bufs=12))
        self.sbrow = ctx.enter_context(tc.tile_pool(name="sbrow", bufs=2))
        self.epool = ctx.enter_context(tc.tile_pool(name="eout", bufs=3))

        # PSUM budget (8 banks): pssw 3 + pstr 2 (pe mode) + ps4 2 + psr 1
        self.pssw = ctx.enter_context(
            tc.tile_pool(name="pssw", bufs=3, space="PSUM"))
        if PH2_MODE == "pe":
            self.pstr = ctx.enter_context(
                tc.tile_pool(name="pstr", bufs=2, space="PSUM"))
        self.ps4 = ctx.enter_context(
            tc.tile_pool(name="ps4", bufs=1, space="PSUM"))
        self.psr = ctx.enter_context(
            tc.tile_pool(name="psr", bufs=1, space="PSUM"))
        self.pools = {"pssw": self.pssw, "vec": self.vec}

    def ph1_load_exp(self, b):
        """DMA load (sync queue) + K = exp(10*m - 40) bf16 + fused rowsum."""
        nc = self.nc
        kt = self.ktp.tile([P, TPM, N], BF16, tag="kt")
        rowsum = self.vec.tile([P, TPM], F32, tag="rs")
        for ti in range(TPM):
            p0 = self.ppool.tile([P, N], F32, tag="p0")
            if ABL_LOAD:
                nc.sync.dma_start(
                    out=p0,
                    in_=self.m_ap[b % self.n_in, ti * P:(ti + 1) * P, :])
            if ABL_EXP:
                nc.scalar.activation(out=kt[:, ti, :], in_=p0, func=AF.Exp,
                                     bias=self.negshift[:, 0:1], scale=INV_EPS,
                                     accum_out=rowsum[:, ti:ti + 1])
            else:
                nc.vector.memset(kt[:, ti, 0:1], 0.001)
                nc.vector.memset(rowsum[:, ti:ti + 1], 1.0)
        u_bf = self.vec.tile([P, TPM], BF16, tag="ubf")
        nc.vector.reciprocal(u_bf, rowsum)
        return dict(b=b, kt=kt, u=u_bf)

    def ph2_transpose(self, m):
        """Build K^T blocks."""
        nc = self.nc
        if not ABL_PH2:
            m["ktt"] = m["kt"]
            return
        ktt = self.kttp.tile([P, TPM, N], BF16, tag="ktt")
        if PH2_MODE == "dma":
            for tj in range(TPM):
                for ti in range(TPM):
                    nc.scalar.dma_start_transpose(
                        out=ktt[:, tj, ti * P:(ti + 1) * P],
                        in_=m["kt"][:, ti, tj * P:(tj + 1) * P])
        else:
            for tj in range(TPM):
                pt = self.pstr.tile([P, N], BF16, tag="pt")
                for ti in range(TPM):
                    nc.tensor.transpose(pt[:, ti * P:(ti + 1) * P],
                                        m["kt"][:, ti, tj * P:(tj + 1) * P],
                                        self.ident_bf)
                if tj % 2 == 0:
                    nc.vector.tensor_copy(ktt[:, tj, :], pt)
                else:
                    nc.scalar.copy(ktt[:, tj, :], pt)
        m["ktt"] = ktt

    def ph3_iterate(self, ms):
        """10 Sinkhorn iterations, pair-interleaved: all u-sweeps of the pair
        first, then all v-sweeps, so each reciprocal hides under the other
        matrix's PE sweep."""
        nc = self.nc
        for t in range(ITERS):
            if t > 0:
                want32 = (t == ITERS - 1)
                for m in ms:
                    u_bf, u32 = _half_sweep(nc, self.pools, m["ktt"], m["v"],
                                            want_f32=want32)
                    if u32 is not None:
                        m["u32"] = u32
                    m["u"] = u_bf
            for m in ms:
                m["v"], _ = _half_sweep(nc, self.pools, m["kt"], m["u"])

    def ph4_output(self, m):
        """out = diag(u32) K diag(v); stores on the SWDGE (gpsimd) queue."""
        nc = self.nc
        if "u32" not in m:
            m["u32"] = m["u"]
        eng = {"gpsimd": nc.gpsimd, "sync": nc.sync,
               "scalar": nc.scalar}[STORE_Q]
        if not ABL_PH4:
            for ti in range(TPM):
                e = self.epool.tile([P, N], F32, tag="e")
                nc.vector.tensor_copy(e[:, 0:8], m["u32"][:, 0:8])
                eng.dma_start(out=self.out_ap[m["b"], ti * P:(ti + 1) * P, :],
                              in_=e)
            return
        vr_ps = self.psr.tile([1, N], BF16, tag="vr", bufs=1)
        for tj in range(TPM):
            nc.tensor.transpose(vr_ps[0:1, tj * P:(tj + 1) * P],
                                m["v"][:, tj:tj + 1], self.ident_bf)
        vrow = self.sbrow.tile([1, N], BF16, tag="vrow")
        nc.vector.tensor_copy(vrow, vr_ps)
        vb = self.ps4.tile([P, N], F32, tag="vb")
        for h in range(2):
            nc.tensor.matmul(vb[:, h * 512:(h + 1) * 512], self.ones_row,
                             vrow[0:1, h * 512:(h + 1) * 512],
                             start=True, stop=True)
        for ti in range(TPM):
            e = self.epool.tile([P, N], F32, tag="e")
            nc.vector.scalar_tensor_tensor(
                out=e, in0=m["kt"][:, ti, :], scalar=m["u32"][:, ti:ti + 1],
                in1=vb, op0=ALU.mult, op1=ALU.mult)
            eng.dma_start(out=self.out_ap[m["b"], ti * P:(ti + 1) * P, :],
                          in_=e)


def sinkhorn_kernel(ctx, tc, out_ap, m_ap, n_in=None):
    nc = tc.nc
    if n_in is None:
        n_in = BPC
    ctx.enter_context(nc.allow_low_precision(
        reason="bf16 u/v iterates validated to 6.5e-3 relmax vs fp64 ref"))
    pipe = _Pipe(ctx, tc, out_ap, m_ap, n_in)

    npairs = BPC // 2
    pairs = [(2 * pr, 2 * pr + 1) for pr in range(npairs)]
    live = [pipe.ph1_load_exp(b) for b in pairs[0]]
    for m in live:
        pipe.ph2_transpose(m)
    for pr in range(npairs):
        nxt = None
        if pr + 1 < npairs:
            # prefetch next pair: loads+exp now (DMA/ACT only)
            nxt = [pipe.ph1_load_exp(b) for b in pairs[pr + 1]]
            if PH2_MODE == "dma":
                for m in nxt:
                    pipe.ph2_transpose(m)
        pipe.ph3_iterate(live)
        if nxt is not None and PH2_MODE == "pe":
            for m in nxt:
                pipe.ph2_transpose(m)
        for m in live:
            pipe.ph4_output(m)
        live = nxt


_CACHE = {}


def _build():
    if "main" in _CACHE:
        return _CACHE["main"]
    nc = bacc.Bacc("TRN2", target_bir_lowering=False, debug=False,
                   num_devices=NCORES)
    m_ap = nc.dram_tensor("m", [BPC, N, N], F32, kind="ExternalInput").ap()
    out_ap = nc.dram_tensor("out", [BPC, N, N], F32, kind="ExternalOutput").ap()
    with tile.TileContext(nc) as tc:
        with ExitStack() as ctx:
            sinkhorn_kernel(ctx, tc, out_ap, m_ap)
    nc.compile()
    _CACHE["main"] = nc
    return nc


def kernel(M: np.ndarray) -> np.ndarray:
    M = np.ascontiguousarray(M, dtype=np.float32)
    assert M.shape == (B, N, N)
    nc = _build()
    in_maps = [{"m": M[c * BPC:(c + 1) * BPC]} for c in range(NCORES)]
    res = run_bass_kernel_spmd(nc, in_maps, core_ids=list(range(NCORES)))
    return np.concatenate([res.results[c]["out"] for c in range(NCORES)], axis=0)


N_IN_TIMING = 2   # aliased timing input matrices (keeps host->device at 8MB)


def _build_timing(loop_n):
    """Timing NEFF: full per-core workload in a hardware For_i loop.
    Input is a small aliased [N_IN_TIMING,N,N] tensor and the output goes to
    internal DRAM scratch, so host<->device transfer noise stays tiny while
    device-side DMA volume per rep is identical to the real kernel."""
    key = ("timing", loop_n)
    if key in _CACHE:
        return _CACHE[key]
    nc = bacc.Bacc("TRN2", target_bir_lowering=False, debug=False,
                   num_devices=NCORES)
    m_ap = nc.dram_tensor("m", [N_IN_TIMING, N, N], F32,
                          kind="ExternalInput").ap()
    out_ap = nc.dram_tensor("oscr", [BPC, N, N], F32, kind="Internal").ap()
    sink_ap = nc.dram_tensor("sink", [P, TPM], F32, kind="ExternalOutput").ap()
    with tile.TileContext(nc) as tc:
        with ExitStack() as ctx:
            with tc.For_i(0, loop_n, 1):
                sinkhorn_kernel(ctx, tc, out_ap, m_ap, n_in=N_IN_TIMING)
        # one tiny real output so the PJRT executable has something to fetch
        with ExitStack() as ctx2:
            pool = ctx2.enter_context(tc.tile_pool(name="snk", bufs=1))
            t = pool.tile([P, TPM], F32)
            nc.vector.memset(t, 1.0)
            nc.sync.dma_start(out=sink_ap, in_=t)
    nc.compile()
    _CACHE[key] = nc
    return nc


def time_hw(lo=100, hi=1100, runs=6, M=None):
    """Per-rep device ns via hardware-loop delta (one rep = full per-core
    workload of BPC matrices; all 8 cores run the same thing in parallel)."""
    import time as _time
    if M is None:
        rng = np.random.default_rng(7)
        M = rng.standard_normal((N_IN_TIMING, N, N), dtype=np.float32)
    in_maps = [{"m": M} for _ in range(NCORES)]
    # Interleave lo/hi samples so tunnel-congestion drift hits both phases
    # equally instead of inflating one side of the delta.
    ncs = {n: _build_timing(n) for n in (lo, hi)}
    for n in (lo, hi):
        run_bass_kernel_spmd(ncs[n], in_maps, core_ids=list(range(NCORES)))
    walls = {lo: [], hi: []}
    for _ in range(runs):
        for n in (lo, hi):
            t0 = _time.time()
            run_bass_kernel_spmd(ncs[n], in_maps, core_ids=list(range(NCORES)))
            walls[n].append(_time.time() - t0)
    for n in (lo, hi):
        print(f"loop_n={n}: walls={[f'{w:.3f}' for w in walls[n]]}", flush=True)
    t = (min(walls[hi]) - min(walls[lo])) / (hi - lo)
    return t * 1e9, walls
